# revision 18
# baseline (speedup 1.0000x reference)
"""AFNO2D layer on 8 TRN2 NeuronCores.

Sharding: channel-block parallel. Core i owns channels [96*i, 96*(i+1)) —
exactly block i of the block-diagonal MLP. The 2D FFT is per-channel and the
MLP is per-block, so there are no cross-core dependencies (no collectives).

Per core, per batch sample (x_c denotes the [h=128, w=128] plane of channel c):
  S1  H-DFT   (mode Q): lhsT=x_c, rhs=[Ch|Sh]                -> psum [w, hk r|i]
  S2  W-rDFT  (mode Q): lhsT=Zt_c, rhs=[Cw|mSw|Sw|Cw]        -> psum [hk, wc r|i]
  C1  pivot: PE-transpose [hk, c] subtiles (fixed wc)        -> Xr/Xi [c, t]
  L1  MLP layer 1 (K=97 bias-row trick) + ReLU               -> O1r/O1i [c, t]
  L2  MLP layer 2 (bias rows hold b2 - lambda) + softshrink  -> O2r/O2i [c, t]
  C2  pivot back: PE-transpose [c, wc] subtiles (fixed hk)   -> Ysp2 [wc, hk*96+c]
  S5  W-irDFT (mode Q): lhsT=Y2_c, rhs=[Cwi|Swi|mSwi|Cwi]    -> psum [hk, w r|i]
  S6  H-iDFT  (mode P): lhsT=[Chi|mShi] const, rhs=T_c       -> psum [h, w] + x_c
"""
import sys
import types
import numpy as np
import ml_dtypes

# run_bass_kernel_spmd(trace=True) needs this hook module; missing in image.
if "antenv.axon_hooks" not in sys.modules:
    _hooks_mod = types.ModuleType("antenv.axon_hooks")
    _hooks_mod._hook = None
    _hooks_mod.set_axon_ntff_profile_hook = lambda h: setattr(_hooks_mod, "_hook", h)
    _hooks_mod.get_axon_ntff_profile_hook = lambda: _hooks_mod._hook
    sys.modules["antenv.axon_hooks"] = _hooks_mod
    try:
        sys.path.insert(0, "/root/.axon_site")
        from trn_agent_boot.trn_boot import _ntff_profile_via_ctypes
        _hooks_mod._hook = _ntff_profile_via_ctypes("/opt/axon/libaxon_pjrt.so")
    except Exception:
        pass

import concourse.bacc as bacc
import concourse.tile as tile
from concourse import mybir
from concourse.bass_utils import run_bass_kernel_spmd

F32 = mybir.dt.float32
BF16 = mybir.dt.bfloat16

B, H, W, C = 4, 128, 128, 768
Wc = W // 2 + 1            # 65
NCORES, BLK = 8, 96        # channels per core
NT = H * Wc                # 8320 tokens per sample
LAM = 0.01

_cache = {}


def _build_consts():
    bf = ml_dtypes.bfloat16
    h = np.arange(H)
    hk = np.arange(H)
    wc = np.arange(Wc)
    w = np.arange(W)
    ang_h = 2 * np.pi * np.outer(h, hk) / H
    Ch, Sh = np.cos(ang_h) / np.sqrt(H), -np.sin(ang_h) / np.sqrt(H)
    ang_w = 2 * np.pi * np.outer(w, wc) / W
    Cw, Sw = np.cos(ang_w) / np.sqrt(W), -np.sin(ang_w) / np.sqrt(W)
    alpha = np.ones(Wc)
    alpha[1:64] = 2.0
    ang_wi = 2 * np.pi * np.outer(wc, w) / W
    Cwi = alpha[:, None] * np.cos(ang_wi) / np.sqrt(W)
    Swi = alpha[:, None] * np.sin(ang_wi) / np.sqrt(W)
    ang_hi = 2 * np.pi * np.outer(hk, h) / H
    Chi, mShi = np.cos(ang_hi) / np.sqrt(H), -np.sin(ang_hi) / np.sqrt(H)

    fh = np.concatenate([Ch, Sh], axis=1).astype(bf)                   # [128,256]
    # fw packed for one accumulation group: ztr @ [Cw|Sw] + zti @ [-Sw|Cw]
    fw = np.concatenate([Cw, Sw, -Sw, Cw], axis=1).astype(bf)          # [128,260]
    # fwi packed likewise: yr @ [Cwi|Swi] + yi @ [-Swi|Cwi]
    fwi = np.concatenate([Cwi, Swi, -Swi, Cwi], axis=1).astype(bf)     # [65,512]
    fhi = np.concatenate([Chi, mShi], axis=1).astype(bf)               # [128,256]
    ident = np.eye(128, dtype=np.float32).astype(bf)                   # [128,128]
    return fh, fw, fwi, fhi, ident


def _pack_mlp(w1, b1, w2, b2, blk):
    """[97, 384] f32 stationary packs: [Wr+br | Wr0 | Wi+bi | -Wi]."""
    def pack(wr, wi, br, bi):
        s = np.zeros((97, 4 * BLK), np.float32)
        s[:BLK, 0:96] = wr
        s[96, 0:96] = br
        s[:BLK, 96:192] = wr
        s[:BLK, 192:288] = wi
        s[96, 192:288] = bi
        s[:BLK, 288:384] = -wi
        return s
    w1s = pack(w1[0, blk], w1[1, blk], b1[0, blk], b1[1, blk])
    w2s = pack(w2[0, blk], w2[1, blk], b2[0, blk] - LAM, b2[1, blk] - LAM)
    return w1s, w2s


def _build_graph(dbg=False):
    nc = bacc.Bacc("TRN2", target_bir_lowering=False, debug=False,
                   num_devices=NCORES)

    x_ext = nc.dram_tensor("x", [B, H, W, BLK], F32, kind="ExternalInput").ap()
    fh_ext = nc.dram_tensor("fh", [128, 256], BF16, kind="ExternalInput").ap()
    fw_ext = nc.dram_tensor("fw", [128, 260], BF16, kind="ExternalInput").ap()
    fwi_ext = nc.dram_tensor("fwi", [65, 512], BF16, kind="ExternalInput").ap()
    fhi_ext = nc.dram_tensor("fhi", [128, 256], BF16, kind="ExternalInput").ap()
    id_ext = nc.dram_tensor("ident", [128, 128], BF16, kind="ExternalInput").ap()
    w1_ext = nc.dram_tensor("w1s", [97, 384], F32, kind="ExternalInput").ap()
    w2_ext = nc.dram_tensor("w2s", [97, 384], F32, kind="ExternalInput").ap()
    out_ext = nc.dram_tensor("out", [B, H, W, BLK], F32, kind="ExternalOutput").ap()
    if dbg:
        d_xr = nc.dram_tensor("d_xr", [97, 512], F32, kind="ExternalOutput").ap()
        d_xi = nc.dram_tensor("d_xi", [97, 512], F32, kind="ExternalOutput").ap()
        d_o1r = nc.dram_tensor("d_o1r", [97, 512], F32, kind="ExternalOutput").ap()
        d_o2r = nc.dram_tensor("d_o2r", [96, 512], F32, kind="ExternalOutput").ap()
        d_w1s = nc.dram_tensor("d_w1s", [97, 384], F32, kind="ExternalOutput").ap()
        d_ysp2 = nc.dram_tensor("d_ysp2", [65, 512], F32, kind="ExternalOutput").ap()
        d_ysp = nc.dram_tensor("d_ysp", [128, 576], F32, kind="ExternalOutput").ap()
        d_zt = nc.dram_tensor("d_zt", [128, 256], F32, kind="ExternalOutput").ap()

    RELU = mybir.ActivationFunctionType.Relu
    SUB = mybir.AluOpType.subtract
    ADD = mybir.AluOpType.add

    with tile.TileContext(nc) as tc:
        with (
            tc.tile_pool(name="consts", bufs=1) as cpool,
            tc.tile_pool(name="bigA", bufs=2) as bigA,     # X32 / Out (24KB bf16)
            tc.tile_pool(name="spec", bufs=1) as spec,     # Ysp2 (48KB)
            tc.tile_pool(name="med", bufs=1) as med,       # Ysp/Xr/Xi/O1/O2
            tc.tile_pool(name="sml", bufs=3) as sml,       # per-channel Zt / T
            tc.tile_pool(name="outc", bufs=2) as outc,
            tc.tile_pool(name="psA", bufs=2, space="PSUM") as psA,   # [128,256] f32
            tc.tile_pool(name="psA2", bufs=2, space="PSUM") as psA2, # [128,130] f32
            tc.tile_pool(name="psTP", bufs=2, space="PSUM") as psTP, # transposes
            tc.tile_pool(name="psB", bufs=2, space="PSUM") as psB,   # [96,512] f32
        ):
            # ---- constants / weights to SBUF (once) ----
            fh = cpool.tile([128, 256], BF16, tag="fh")
            nc.sync.dma_start(out=fh, in_=fh_ext)
            fw = cpool.tile([128, 260], BF16, tag="fw")
            nc.sync.dma_start(out=fw, in_=fw_ext)
            fwi = cpool.tile([65, 512], BF16, tag="fwi")
            nc.sync.dma_start(out=fwi, in_=fwi_ext)
            fhi = cpool.tile([128, 256], BF16, tag="fhi")
            nc.sync.dma_start(out=fhi, in_=fhi_ext)
            ident = cpool.tile([128, 128], BF16, tag="ident")
            nc.sync.dma_start(out=ident, in_=id_ext)
            w1s = cpool.tile([97, 384], BF16, tag="w1s")
            nc.gpsimd.dma_start(out=w1s, in_=w1_ext)      # casting DMA f32->bf16
            w2s = cpool.tile([97, 384], BF16, tag="w2s")
            nc.gpsimd.dma_start(out=w2s, in_=w2_ext)
            n2lam = cpool.tile([128, 1], F32, tag="n2lam")
            nc.vector.memset(n2lam[:], -2.0 * LAM)

            W1 = {k: w1s[:, i * 96:(i + 1) * 96] for i, k in
                  enumerate(("rb", "r0", "ib", "mi"))}
            W2 = {k: w2s[:, i * 96:(i + 1) * 96] for i, k in
                  enumerate(("rb", "r0", "ib", "mi"))}

            for b in range(B):
                # ---- load sample: X32 [h, (w,c)] bf16 via casting DMA ----
                X32 = bigA.tile([128, W, BLK], BF16, tag="bigA")
                nc.gpsimd.dma_start(out=X32, in_=x_ext[b])

                Ysp = med.tile([128, Wc, BLK, 2], BF16, tag="medY")  # [hk,(wc,c,ri)]

                # ---- S1 + S2 per channel ----
                for c in range(BLK):
                    zt = sml.tile([128, 256], BF16, tag="zt")
                    p1 = psA.tile([128, 256], F32, tag="psA")
                    nc.tensor.matmul(p1[:], X32[:, :, c], fh[:], start=True, stop=True)
                    # split drain between DVE and ACT
                    nc.vector.tensor_copy(zt[:, 0:128], p1[:, 0:128])
                    nc.scalar.copy(zt[:, 128:256], p1[:, 128:256])

                    p2 = psA2.tile([128, 130], F32, tag="psA2")
                    ztr, zti = zt[:, 0:128], zt[:, 128:256]
                    nc.tensor.matmul(p2[:], ztr, fw[:, 0:130], start=True, stop=False)
                    nc.tensor.matmul(p2[:], zti, fw[:, 130:260], start=False, stop=True)
                    # Yr|Yi [hk, 65] -> Ysp[:, :, c, :]
                    nc.scalar.copy(Ysp[:, :, c, 0], p2[:, 0:65])
                    nc.scalar.copy(Ysp[:, :, c, 1], p2[:, 65:130])
                    if dbg and b == 0 and c == 0:
                        nc.gpsimd.dma_start(out=d_zt, in_=zt[:])

                if dbg and b == 0:
                    nc.gpsimd.dma_start(
                        out=d_ysp,
                        in_=Ysp.rearrange("p a b r -> p (a b r)")[:, 0:576])

                # ---- C1: pivot to channel-major [c, t], t = hk*65+wc ----
                Xr = med.tile([97, NT], BF16, tag="medXr")
                Xi = med.tile([97, NT], BF16, tag="medXi")
                nc.vector.memset(Xr[96:97, :], 1.0)
                nc.vector.memset(Xi[96:97, :], 1.0)
                Xr3 = Xr.rearrange("p (hk wc) -> p hk wc", wc=Wc)
                Xi3 = Xi.rearrange("p (hk wc) -> p hk wc", wc=Wc)
                for wcc in range(Wc):
                    ptr = psTP.tile([96, 128], BF16, tag="psTP")
                    nc.tensor.transpose(ptr[:], Ysp[:, wcc, :, 0], ident[:])
                    nc.vector.tensor_copy(Xr3[0:96, :, wcc], ptr[:])
                    pti = psTP.tile([96, 128], BF16, tag="psTP")
                    nc.tensor.transpose(pti[:], Ysp[:, wcc, :, 1], ident[:])
                    nc.vector.tensor_copy(Xi3[0:96, :, wcc], pti[:])

                if dbg and b == 0:
                    nc.gpsimd.dma_start(out=d_xr, in_=Xr[:, 0:512])
                    nc.gpsimd.dma_start(out=d_xi, in_=Xi[:, 0:512])

                # ---- MLP ----
                O1r = med.tile([97, NT], BF16, tag="medO1r")
                O1i = med.tile([97, NT], BF16, tag="medO1i")
                nc.vector.memset(O1r[96:97, :], 1.0)
                nc.vector.memset(O1i[96:97, :], 1.0)
                O2r = med.tile([96, NT], BF16, tag="medXr")
                O2i = med.tile([96, NT], BF16, tag="medXi")
                nchunks = (NT + 511) // 512
                for k in range(nchunks):
                    sl = slice(k * 512, min((k + 1) * 512, NT))
                    n = sl.stop - sl.start
                    pr = psB.tile([96, 512], F32, tag="psB")
                    pi = psB.tile([96, 512], F32, tag="psB")
                    nc.tensor.matmul(pr[:, :n], W1["rb"], Xr[:, sl], start=True, stop=False)
                    nc.tensor.matmul(pr[:, :n], W1["mi"], Xi[:, sl], start=False, stop=True)
                    nc.tensor.matmul(pi[:, :n], W1["ib"], Xr[:, sl], start=True, stop=False)
                    nc.tensor.matmul(pi[:, :n], W1["r0"], Xi[:, sl], start=False, stop=True)
                    nc.scalar.activation(O1r[0:96, sl], pr[:, :n], RELU)
                    nc.scalar.activation(O1i[0:96, sl], pi[:, :n], RELU)

                for k in range(nchunks):
                    sl = slice(k * 512, min((k + 1) * 512, NT))
                    n = sl.stop - sl.start
                    pr2 = psB.tile([96, 512], F32, tag="psB")
                    pi2 = psB.tile([96, 512], F32, tag="psB")
                    nc.tensor.matmul(pr2[:, :n], W2["rb"], O1r[:, sl], start=True, stop=False)
                    nc.tensor.matmul(pr2[:, :n], W2["mi"], O1i[:, sl], start=False, stop=True)
                    nc.tensor.matmul(pi2[:, :n], W2["ib"], O1r[:, sl], start=True, stop=False)
                    nc.tensor.matmul(pi2[:, :n], W2["r0"], O1i[:, sl], start=False, stop=True)
                    # softshrink(v) = relu(v-lam) - relu(-(v-lam) - 2lam); psum holds v-lam
                    for psv, O2 in ((pr2, O2r), (pi2, O2i)):
                        s_a = outc.tile([96, 512], BF16, tag="sa")
                        s_b = outc.tile([96, 512], BF16, tag="sb")
                        nc.scalar.activation(s_a[:, :n], psv[:, :n], RELU)
                        nc.vector.tensor_scalar(s_b[:, :n], psv[:, :n], -1.0, None,
                                                mybir.AluOpType.mult)
                        nc.scalar.activation(s_b[:, :n], s_b[:, :n], RELU,
                                             bias=n2lam[0:96])
                        nc.gpsimd.tensor_tensor(O2[:, sl], s_a[:, :n], s_b[:, :n], SUB)

                if dbg and b == 0:
                    nc.gpsimd.dma_start(out=d_o1r, in_=O1r[:, 0:512])
                    nc.gpsimd.dma_start(out=d_o2r, in_=O2r[:, 0:512])
                    nc.gpsimd.dma_start(out=d_w1s, in_=w1s[:])

                # ---- C2: pivot back, Ysp2 [wc, (hk, c)] ----
                Ysp2 = spec.tile([65, H, BLK, 2], BF16, tag="spec")
                O2r3 = O2r.rearrange("p (hk wc) -> p hk wc", wc=Wc)
                O2i3 = O2i.rearrange("p (hk wc) -> p hk wc", wc=Wc)
                for hkk in range(H):
                    ptr = psTP.tile([65, 96], BF16, tag="psTP")
                    nc.tensor.transpose(ptr[:], O2r3[:, hkk, :], ident[0:96, 0:96])
                    nc.vector.tensor_copy(Ysp2[:, hkk, :, 0], ptr[:])
                    pti = psTP.tile([65, 96], BF16, tag="psTP")
                    nc.tensor.transpose(pti[:], O2i3[:, hkk, :], ident[0:96, 0:96])
                    nc.vector.tensor_copy(Ysp2[:, hkk, :, 1], pti[:])

                if dbg and b == 0:
                    nc.gpsimd.dma_start(
                        out=d_ysp2,
                        in_=Ysp2.rearrange("p a b r -> p (a b r)")[:, 0:512])

                # ---- S5 + S6 per channel ----
                Out = bigA.tile([128, W, BLK], BF16, tag="bigA")
                for c in range(BLK):
                    p5 = psA.tile([128, 256], F32, tag="psA")
                    yr = Ysp2[:, :, c, 0]                       # [65, 128]
                    yi = Ysp2[:, :, c, 1]
                    nc.tensor.matmul(p5[:], yr, fwi[:, 0:256], start=True, stop=False)
                    nc.tensor.matmul(p5[:], yi, fwi[:, 256:512], start=False, stop=True)
                    tt = sml.tile([128, 256], BF16, tag="tt")
                    nc.vector.tensor_copy(tt[:, 0:128], p5[:, 0:128])
                    nc.scalar.copy(tt[:, 128:256], p5[:, 128:256])

                    p6 = psA2.tile([128, 128], F32, tag="psA2")
                    nc.tensor.matmul(p6[:], fhi[:, 0:128], tt[:, 0:128], start=True, stop=False)
                    nc.tensor.matmul(p6[:], fhi[:, 128:256], tt[:, 128:256], start=False, stop=True)
                    nc.vector.tensor_tensor(Out[:, :, c], p6[:], X32[:, :, c], ADD)

                nc.gpsimd.dma_start(out=out_ext[b], in_=Out)   # casting DMA bf16->f32

    nc.compile()
    return nc


def kernel(x, w1, b1, w2, b2):
    x = np.ascontiguousarray(x, dtype=np.float32)
    key = "nc"
    if key not in _cache:
        _cache[key] = _build_graph()
    nc = _cache[key]

    fh, fw, fwi, fhi, ident = _build_consts()
    in_maps = []
    for i in range(NCORES):
        w1s, w2s = _pack_mlp(w1, b1, w2, b2, i)
        in_maps.append({
            "x": np.ascontiguousarray(x[:, :, :, i * BLK:(i + 1) * BLK]),
            "fh": fh, "fw": fw, "fwi": fwi, "fhi": fhi, "ident": ident,
            "w1s": w1s, "w2s": w2s,
        })
    res = run_bass_kernel_spmd(nc, in_maps, core_ids=list(range(NCORES)))
    out = np.concatenate([res.results[i]["out"] for i in range(NCORES)], axis=3)
    return out.astype(np.float32)


# revision 26
# speedup vs baseline: 1.5893x; 1.5893x over previous
"""AFNO2D layer on 8 TRN2 NeuronCores.

Sharding: channel-block parallel. Core i owns channels [96*i, 96*(i+1)) —
exactly block i of the block-diagonal MLP. The 2D FFT is per-channel and the
MLP is per-block, so there are no cross-core dependencies (no collectives).

Per core, per batch sample:
  S1  H-DFT  (mode Q, per channel c): lhsT=x_c [h,w], rhs=[Ch|Sh]
      -> psum [w, hk r|i] -> Zt [w, c, hk r|i]
  S2  W-rDFT (mode Q, per row hk): lhsT=Zt[:, :, hk] (strided [w, c]),
      rhs=[Cw|Sw | -Sw|Cw] -> psum [c, wc r|i] -> Xr/Xi [c, t=hk*65+wc]
      (the c-pivot falls out of mode Q for free; no transposes needed)
  L1  MLP layer 1; bias+ReLU fused into the ACT drain        -> O1r/O1i [c, t]
  L2  MLP layer 2; bias and softshrink fused into the drains -> O2r/O2i [c, t]
  C2  pivot back: PE-transpose [c, wc] subtiles (fixed hk)   -> Ysp2 [wc, hk*96+c]
  S5  W-irDFT (mode Q, per c): lhsT=Y2_c, rhs=[Cwi|Swi|-Swi|Cwi] -> [hk, w r|i]
  S6  H-iDFT (mode P, const lhsT=[Chi|-Shi], 2 channels/matmul) -> corr [h, w]
The residual add (out = corr + x) runs on the host in fp32.
"""
import sys
import types
import numpy as np
import ml_dtypes

# run_bass_kernel_spmd(trace=True) needs this hook module; missing in image.
if "antenv.axon_hooks" not in sys.modules:
    _hooks_mod = types.ModuleType("antenv.axon_hooks")
    _hooks_mod._hook = None
    _hooks_mod.set_axon_ntff_profile_hook = lambda h: setattr(_hooks_mod, "_hook", h)
    _hooks_mod.get_axon_ntff_profile_hook = lambda: _hooks_mod._hook
    sys.modules["antenv.axon_hooks"] = _hooks_mod
    try:
        sys.path.insert(0, "/root/.axon_site")
        from trn_agent_boot.trn_boot import _ntff_profile_via_ctypes
        _hooks_mod._hook = _ntff_profile_via_ctypes("/opt/axon/libaxon_pjrt.so")
    except Exception:
        pass

import concourse.bacc as bacc
import concourse.tile as tile
from concourse import mybir
from concourse.bass_utils import run_bass_kernel_spmd

F32 = mybir.dt.float32
BF16 = mybir.dt.bfloat16

B, H, W, C = 4, 128, 128, 768
Wc = W // 2 + 1            # 65
NCORES, BLK = 8, 96        # channels per core
NT = H * Wc                # 8320 tokens per sample
LAM = 0.01

_cache = {}


def _build_consts():
    bf = ml_dtypes.bfloat16
    h = np.arange(H)
    hk = np.arange(H)
    wc = np.arange(Wc)
    w = np.arange(W)
    ang_h = 2 * np.pi * np.outer(h, hk) / H
    Ch, Sh = np.cos(ang_h) / np.sqrt(H), -np.sin(ang_h) / np.sqrt(H)
    ang_w = 2 * np.pi * np.outer(w, wc) / W
    Cw, Sw = np.cos(ang_w) / np.sqrt(W), -np.sin(ang_w) / np.sqrt(W)
    alpha = np.ones(Wc)
    alpha[1:64] = 2.0
    ang_wi = 2 * np.pi * np.outer(wc, w) / W
    Cwi = alpha[:, None] * np.cos(ang_wi) / np.sqrt(W)
    Swi = alpha[:, None] * np.sin(ang_wi) / np.sqrt(W)
    ang_hi = 2 * np.pi * np.outer(hk, h) / H
    Chi, mShi = np.cos(ang_hi) / np.sqrt(H), -np.sin(ang_hi) / np.sqrt(H)

    fh = np.concatenate([Ch, Sh], axis=1).astype(bf)                   # [128,256]
    # one accumulation group: ztr @ [Cw|Sw] + zti @ [-Sw|Cw]
    fw = np.concatenate([Cw, Sw, -Sw, Cw], axis=1).astype(bf)          # [128,260]
    # one accumulation group: yr @ [Cwi|Swi] + yi @ [-Swi|Cwi]
    fwi = np.concatenate([Cwi, Swi, -Swi, Cwi], axis=1).astype(bf)     # [65,512]
    fhi = np.concatenate([Chi, mShi], axis=1).astype(bf)               # [128,256]
    ident = np.eye(96, dtype=np.float32).astype(bf)                    # [96,96]
    return fh, fw, fwi, fhi, ident


def _pack_mlp(w1, b1, w2, b2, blk):
    """[96, 384] weight packs [Wr | Wr | Wi | -Wi]; [96, 6] bias vectors."""
    def pack(wr, wi):
        return np.concatenate([wr, wr, wi, -wi], axis=1).astype(np.float32)
    w1s = pack(w1[0, blk], w1[1, blk])
    w2s = pack(w2[0, blk], w2[1, blk])
    bv = np.stack([
        b1[0, blk], b1[1, blk],
        b2[0, blk] - LAM, -b2[0, blk] - LAM,
        b2[1, blk] - LAM, -b2[1, blk] - LAM,
    ], axis=1).astype(np.float32)                                      # [96, 6]
    return w1s, w2s, bv


def _build_graph():
    nc = bacc.Bacc("TRN2", target_bir_lowering=False, debug=False,
                   num_devices=NCORES)

    x_ext = nc.dram_tensor("x", [B, H, W, BLK], F32, kind="ExternalInput").ap()
    fh_ext = nc.dram_tensor("fh", [128, 256], BF16, kind="ExternalInput").ap()
    fw_ext = nc.dram_tensor("fw", [128, 260], BF16, kind="ExternalInput").ap()
    fwi_ext = nc.dram_tensor("fwi", [65, 512], BF16, kind="ExternalInput").ap()
    fhi_ext = nc.dram_tensor("fhi", [128, 256], BF16, kind="ExternalInput").ap()
    id_ext = nc.dram_tensor("ident", [96, 96], BF16, kind="ExternalInput").ap()
    w1_ext = nc.dram_tensor("w1s", [96, 384], F32, kind="ExternalInput").ap()
    w2_ext = nc.dram_tensor("w2s", [96, 384], F32, kind="ExternalInput").ap()
    bv_ext = nc.dram_tensor("bv", [96, 6], F32, kind="ExternalInput").ap()
    # [B, H, c-pair, W, 2] so each (h, pair) row is one contiguous 512B run;
    # the host permutes back to [B, H, W, BLK].
    out_ext = nc.dram_tensor("out", [B, H, BLK // 2, W, 2], BF16,
                             kind="ExternalOutput").ap()

    RELU = mybir.ActivationFunctionType.Relu
    SUB = mybir.AluOpType.subtract

    with tile.TileContext(nc) as tc:
        with (
            tc.tile_pool(name="consts", bufs=1) as cpool,
            tc.tile_pool(name="bigA", bufs=1) as bigA,     # X32 (24KB bf16)
            tc.tile_pool(name="zsp", bufs=2) as zsp,       # Zt / Ysp2 (48KB slots)
            tc.tile_pool(name="med", bufs=1) as med,       # Xr/Xi/O1r/O1i (+O2 reuse)
            tc.tile_pool(name="sml", bufs=3) as sml,       # per-channel-pair T tiles
            tc.tile_pool(name="outc", bufs=3) as outc,
            tc.tile_pool(name="psA", bufs=2, space="PSUM") as psA,    # S1/S5 [128,256]
            tc.tile_pool(name="psA2", bufs=2, space="PSUM") as psA2,  # S2'/S6
            tc.tile_pool(name="psTP", bufs=2, space="PSUM") as psTP,  # C2 transposes
            tc.tile_pool(name="psB", bufs=2, space="PSUM") as psB,    # MLP [96,512]
        ):
            # ---- constants / weights to SBUF (once) ----
            fh = cpool.tile([128, 256], BF16, tag="fh")
            nc.sync.dma_start(out=fh, in_=fh_ext)
            fw = cpool.tile([128, 260], BF16, tag="fw")
            nc.sync.dma_start(out=fw, in_=fw_ext)
            fwi = cpool.tile([65, 512], BF16, tag="fwi")
            nc.sync.dma_start(out=fwi, in_=fwi_ext)
            fhi = cpool.tile([128, 256], BF16, tag="fhi")
            nc.sync.dma_start(out=fhi, in_=fhi_ext)
            ident = cpool.tile([96, 96], BF16, tag="ident")
            nc.sync.dma_start(out=ident, in_=id_ext)
            w1s = cpool.tile([96, 384], BF16, tag="w1s")
            nc.gpsimd.dma_start(out=w1s, in_=w1_ext)      # casting DMA f32->bf16
            w2s = cpool.tile([96, 384], BF16, tag="w2s")
            nc.gpsimd.dma_start(out=w2s, in_=w2_ext)
            bv = cpool.tile([96, 6], F32, tag="bv")
            nc.sync.dma_start(out=bv, in_=bv_ext)

            W1 = {k: w1s[:, i * 96:(i + 1) * 96] for i, k in
                  enumerate(("ra", "rb", "ib", "mi"))}
            W2 = {k: w2s[:, i * 96:(i + 1) * 96] for i, k in
                  enumerate(("ra", "rb", "ib", "mi"))}
            b1r, b1i = bv[:, 0:1], bv[:, 1:2]
            b2ra, b2rb = bv[:, 2:3], bv[:, 3:4]
            b2ia, b2ib = bv[:, 4:5], bv[:, 5:6]

            for b in range(B):
                # ---- load sample: X32 [h, (w,c)] bf16 via casting DMA ----
                X32 = bigA.tile([128, W, BLK], BF16, tag="bigA")
                nc.gpsimd.dma_start(out=X32, in_=x_ext[b])

                # ---- S1 per channel -> Zt [w, c, hk r|i] ----
                Zt = zsp.tile([128, BLK, 256], BF16, tag="zsp")
                for c in range(BLK):
                    p1 = psA.tile([128, 256], F32, tag="psA")
                    nc.tensor.matmul(p1[:], X32[:, :, c], fh[:], start=True, stop=True)
                    nc.vector.tensor_copy(Zt[:, c, 0:128], p1[:, 0:128])
                    nc.scalar.copy(Zt[:, c, 128:256], p1[:, 128:256])

                # ---- S2' per hk row -> Xr/Xi [c, t=hk*65+wc] directly ----
                Xr = med.tile([96, NT], BF16, tag="medXr")
                Xi = med.tile([96, NT], BF16, tag="medXi")
                for hk in range(H):
                    p2 = psA2.tile([96, 130], F32, tag="psA2")
                    ztr = Zt[:, :, hk]              # [w=128, c=96] stride-256
                    zti = Zt[:, :, 128 + hk]
                    nc.tensor.matmul(p2[:], ztr, fw[:, 0:130], start=True, stop=False)
                    nc.tensor.matmul(p2[:], zti, fw[:, 130:260], start=False, stop=True)
                    sl = slice(hk * Wc, (hk + 1) * Wc)
                    if hk % 2 == 0:
                        nc.vector.tensor_copy(Xr[:, sl], p2[:, 0:65])
                        nc.scalar.copy(Xi[:, sl], p2[:, 65:130])
                    else:
                        nc.scalar.copy(Xr[:, sl], p2[:, 0:65])
                        nc.vector.tensor_copy(Xi[:, sl], p2[:, 65:130])

                # ---- MLP ----
                O1r = med.tile([96, NT], BF16, tag="medO1r")
                O1i = med.tile([96, NT], BF16, tag="medO1i")
                nchunks = (NT + 511) // 512
                for k in range(nchunks):
                    sl = slice(k * 512, min((k + 1) * 512, NT))
                    n = sl.stop - sl.start
                    pr = psB.tile([96, 512], F32, tag="psB")
                    pi = psB.tile([96, 512], F32, tag="psB")
                    nc.tensor.matmul(pr[:, :n], W1["ra"], Xr[:, sl], start=True, stop=False)
                    nc.tensor.matmul(pr[:, :n], W1["mi"], Xi[:, sl], start=False, stop=True)
                    nc.tensor.matmul(pi[:, :n], W1["ib"], Xr[:, sl], start=True, stop=False)
                    nc.tensor.matmul(pi[:, :n], W1["rb"], Xi[:, sl], start=False, stop=True)
                    nc.scalar.activation(O1r[:, sl], pr[:, :n], RELU, bias=b1r)
                    nc.scalar.activation(O1i[:, sl], pi[:, :n], RELU, bias=b1i)

                O2r = med.tile([96, NT], BF16, tag="medXr")
                O2i = med.tile([96, NT], BF16, tag="medXi")
                for k in range(nchunks):
                    sl = slice(k * 512, min((k + 1) * 512, NT))
                    n = sl.stop - sl.start
                    pr2 = psB.tile([96, 512], F32, tag="psB")
                    pi2 = psB.tile([96, 512], F32, tag="psB")
                    nc.tensor.matmul(pr2[:, :n], W2["ra"], O1r[:, sl], start=True, stop=False)
                    nc.tensor.matmul(pr2[:, :n], W2["mi"], O1i[:, sl], start=False, stop=True)
                    nc.tensor.matmul(pi2[:, :n], W2["ib"], O1r[:, sl], start=True, stop=False)
                    nc.tensor.matmul(pi2[:, :n], W2["rb"], O1i[:, sl], start=False, stop=True)
                    # softshrink(v) = relu(v + b - lam) - relu(-v - b - lam)
                    for psv, O2, ba, bb in ((pr2, O2r, b2ra, b2rb),
                                            (pi2, O2i, b2ia, b2ib)):
                        s_a = outc.tile([96, 512], BF16, tag="sa")
                        s_b = outc.tile([96, 512], BF16, tag="sb")
                        nc.scalar.activation(s_a[:, :n], psv[:, :n], RELU, bias=ba)
                        nc.scalar.activation(s_b[:, :n], psv[:, :n], RELU,
                                             bias=bb, scale=-1.0)
                        nc.gpsimd.tensor_tensor(O2[:, sl], s_a[:, :n], s_b[:, :n], SUB)

                # ---- C2: pivot back, Ysp2 [wc, (hk, c)] ----
                Ysp2 = zsp.tile([65, H, BLK, 2], BF16, tag="zsp")
                O2r3 = O2r.rearrange("p (hk wc) -> p hk wc", wc=Wc)
                O2i3 = O2i.rearrange("p (hk wc) -> p hk wc", wc=Wc)
                for hkk in range(H):
                    ptr = psTP.tile([65, 96], BF16, tag="psTP")
                    nc.tensor.transpose(ptr[:], O2r3[:, hkk, :], ident[:])
                    nc.vector.tensor_copy(Ysp2[:, hkk, :, 0], ptr[:])
                    pti = psTP.tile([65, 96], BF16, tag="psTP")
                    nc.tensor.transpose(pti[:], O2i3[:, hkk, :], ident[:])
                    nc.vector.tensor_copy(Ysp2[:, hkk, :, 1], pti[:])

                # ---- S5 (per c) + S6 (per channel pair) ----
                for c0 in range(0, BLK, 2):
                    tt = sml.tile([128, 2, 2, 128], BF16, tag="tt")  # [c2, ri, w]
                    for j in (0, 1):
                        c = c0 + j
                        p5 = psA.tile([128, 256], F32, tag="psA")
                        yr = Ysp2[:, :, c, 0]                       # [65, 128]
                        yi = Ysp2[:, :, c, 1]
                        nc.tensor.matmul(p5[:], yr, fwi[:, 0:256], start=True, stop=False)
                        nc.tensor.matmul(p5[:], yi, fwi[:, 256:512], start=False, stop=True)
                        nc.vector.tensor_copy(tt[:, j, 0, :], p5[:, 0:128])
                        nc.scalar.copy(tt[:, j, 1, :], p5[:, 128:256])

                    p6 = psA2.tile([128, 2, 128], F32, tag="psA2")
                    nc.tensor.matmul(p6[:], fhi[:, 0:128], tt[:, :, 0, :],
                                     start=True, stop=False)
                    nc.tensor.matmul(p6[:], fhi[:, 128:256], tt[:, :, 1, :],
                                     start=False, stop=True)
                    oc = outc.tile([128, 128, 2], BF16, tag="oc")
                    nc.vector.tensor_copy(oc[:, :, 0], p6[:, 0, :])
                    nc.scalar.copy(oc[:, :, 1], p6[:, 1, :])
                    nc.sync.dma_start(out=out_ext[b, :, c0 // 2], in_=oc)

    nc.compile()
    return nc


def kernel(x, w1, b1, w2, b2):
    x = np.ascontiguousarray(x, dtype=np.float32)
    key = "nc"
    if key not in _cache:
        _cache[key] = _build_graph()
    nc = _cache[key]

    in_maps = make_in_maps(x, w1, b1, w2, b2)
    res = run_bass_kernel_spmd(nc, in_maps, core_ids=list(range(NCORES)))
    # device layout [B, H, pair, W, 2] -> [B, H, W, BLK]
    corr = np.concatenate(
        [np.asarray(res.results[i]["out"], dtype=np.float32)
         .transpose(0, 1, 3, 2, 4).reshape(B, H, W, BLK)
         for i in range(NCORES)], axis=3)
    return (corr + x).astype(np.float32)


def make_in_maps(x, w1, b1, w2, b2):
    fh, fw, fwi, fhi, ident = _build_consts()
    in_maps = []
    for i in range(NCORES):
        w1s, w2s, bv = _pack_mlp(w1, b1, w2, b2, i)
        in_maps.append({
            "x": np.ascontiguousarray(x[:, :, :, i * BLK:(i + 1) * BLK]),
            "fh": fh, "fw": fw, "fwi": fwi, "fhi": fhi, "ident": ident,
            "w1s": w1s, "w2s": w2s, "bv": bv,
        })
    return in_maps
